# revision 1
# baseline (speedup 1.0000x reference)
"""Trainium2 8-core SPMD kernel for MQA attention with relative position bias.

Reference computation (b=2, n=2048, D=1024, h=8, dh=64, MQA single k/v head):
    q  = x @ Wq;  kv = x @ Wkv;  k, v = kv[..., :64], kv[..., 64:]
    sim = (q[b,h,i,:] . k[b,j,:]) * dh**-0.5 + rel_pos_bias[h,i,j]   (causal masked)
    out = softmax(sim) @ v  -> reshape -> @ Wo + bo

v3 design (collective-free, pipelined):
  - Queries are sharded across the 8 cores. Core c owns q-tiles {c, 15-c}
    of each batch (128 tokens per tile -> 512 tokens/core), which balances
    causal work exactly. All 8 heads are computed on every core (MQA).
  - NO collectives: every core receives the full x (bf16, transposed) and
    computes the (tiny) k/v projection for all 4096 tokens redundantly.
    Each core's NEFF is fully independent -- no cross-core rendezvous, so
    per-core launch skew cannot inflate the executed span.
  - x streams in 512-token windows; the fused k||v projection (one M=128
    stationary Wkv) chases the DMA window by window. v is re-oriented
    token-major with PE transposes; a ones-column per V65 tile yields the
    softmax denominator via the M=65 PV matmul trick.
  - Scores are computed transposed (keys on partitions, queries free), 4
    heads per N=512 matmul (3D rhs AP on q). The rel-pos bias is applied
    multiplicatively AFTER the exp: exp(qk/8 + b) = exp(qk/8) * exp(b),
    with exp(b) precomputed on the host (bf16; causal-masked entries are
    exactly 0, so masking rides along). The multiply is a pure-SBUF bf16
    DVE op (4x SIMD mode) -- GPSIMD cannot touch PSUM on TRN2, and this
    keeps every PSUM reader on DVE/ACT/PE.
  - exp on ScalarE only (Exp/Ln share one table set; all copies live on
    DVE/Pool to avoid table thrash). PV = 2 N=512 bf16 matmuls per pair
    (MQA heads are just more moving columns; no 3D AP needed).
  - The pair loop is software-pipelined (scores of pair i+1 are emitted
    before PV of pair i) to keep the PE dense (p-state ramp).
  - Output projection is row-sharded; bias/out DMAs ride the second
    HWDGE queue (ACT) so the SP queue streams x/weights uninterrupted.
"""

import os
import sys

import numpy as np

sys.path.insert(0, "/opt/trn_rl_repo")

import ml_dtypes

BF16 = ml_dtypes.bfloat16

# ---- problem constants (hardcoded per the harness contract) ----
B = 2
N = 2048
DIM = 1024
HEADS = 8
DH = 64
INNER = HEADS * DH  # 512
P = 128
NT = N // P  # 16 q/k tiles per batch
EXTA, EXTB = 8, 16  # j-tile extents for slot A (q-tile c) / slot B (q-tile 15-c)
NPAIR = EXTA + EXTB  # 24 (slot, j-tile) pairs per batch per core
NCORES = 8
TOK_OWN = 4 * P  # 512 own tokens per core
TOK_ALL = B * N  # 4096
NWIN = TOK_ALL // 512  # 8 kv windows
NEG = -1.0e30  # masked logit (pre-scale), exp -> 0

_CACHE = {}


def _q_tiles(c):
    return [c, NT - 1 - c]


def build_graph(
    use_cc=False,  # ignored (kept for simbench compat); v3 has no collective
    dve_cols=640,  # bias-add split: DVE takes [0:dve_cols], Pool the rest
    st_bufs=2,
    pv_bufs=2,
    lg_bufs=3,
    tail_at=7,  # flush previous slot's tail after this pair of the next slot
    reps=1,
):
    import concourse.bass as bass
    import concourse.bacc as bacc
    import concourse.mybir as mybir
    import concourse.tile as tile

    dt = mybir.dt
    f32, f32r, bf16 = dt.float32, dt.float32r, dt.bfloat16
    AF = mybir.ActivationFunctionType

    nc = bacc.Bacc(None, target_bir_lowering=False)

    # ---- I/O ----
    xTb_t = nc.dram_tensor("xTb", [DIM, TOK_ALL], bf16, kind="ExternalInput")
    xTo_t = nc.dram_tensor("xTo", [DIM, TOK_OWN], bf16, kind="ExternalInput")
    Wq_t = nc.dram_tensor("Wq", [DIM, INNER], bf16, kind="ExternalInput")
    Wkv_t = nc.dram_tensor("Wkv", [DIM, 2 * DH], bf16, kind="ExternalInput")
    Wo_t = nc.dram_tensor("Wo", [INNER, DIM], bf16, kind="ExternalInput")
    bo_t = nc.dram_tensor("bo", [1, DIM], f32r, kind="ExternalInput")
    ident_t = nc.dram_tensor("ident", [P, DH], bf16, kind="ExternalInput")
    ones_r_t = nc.dram_tensor("ones_r", [1, P], f32r, kind="ExternalInput")
    # biasT[b, pair, j, h, q]: transposed, causal-masked, x8-scaled bias
    bias_t = nc.dram_tensor(
        "biasT", [B, NPAIR, P, HEADS, P], bf16, kind="ExternalInput"
    )
    out_t = nc.dram_tensor("out", [TOK_OWN, DIM], f32, kind="ExternalOutput")

    with tile.TileContext(nc) as tc:
        with (
            tc.tile_pool(name="const", bufs=1) as cpool,
            tc.tile_pool(name="bias", bufs=5) as bpool,
            tc.tile_pool(name="pt", bufs=4) as ptpool,
            tc.tile_pool(name="at", bufs=3) as atpool,
            tc.tile_pool(name="ob", bufs=3) as obpool,
            tc.tile_pool(name="ps", bufs=1, space="PSUM") as pspool,
        ):
            # ---- small constants / weights first (SP queue) ----
            # one DMA per tensor: chunk-folded [p, c, t] APs (HWDGE costs
            # ~625ns per DMA on a single shared resource -- count matters)
            Wkv_sb = cpool.tile([P, 8 * 2 * DH], bf16, tag="Wkv_sb")
            nc.sync.dma_start(
                out=Wkv_sb[:].rearrange("p (c d) -> p c d", c=8),
                in_=Wkv_t.rearrange("(c p) d -> p c d", p=P),
            )
            # identity at both partition halves (transpose lhsT sits at 64:128)
            ident_sb = cpool.tile([P, DH], bf16, tag="ident_sb")
            nc.sync.dma_start(out=ident_sb[:], in_=ident_t[:])
            ones128 = cpool.tile([1, P], f32r, tag="ones128")
            nc.sync.dma_start(out=ones128[:], in_=ones_r_t[:])
            bo_sb = cpool.tile([1, DIM], f32r, tag="bo_sb")
            nc.sync.dma_start(out=bo_sb[:], in_=bo_t[:])
            # preload the Exp table so the first stream exp pays no
            # LoadActFuncSet on the critical path
            scr = cpool.tile([1, 4], f32, tag="scr")
            nc.scalar.activation(scr[:, :], ones128[:, 0:4], AF.Exp)
            # full x in 3 window-group DMAs so kv can chase the stream; bias
            # prefetch DMAs are interleaved (the DMA device drains in order)
            xTb_sb = cpool.tile([P, 8 * TOK_ALL], bf16, tag="xTb_sb")
            xTo_sb = cpool.tile([P, 8 * TOK_OWN], bf16, tag="xTo_sb")
            Wq_sb = cpool.tile([P, 8 * INNER], bf16, tag="Wq_sb")
            Wo_sb = cpool.tile([P, 4 * DIM], bf16, tag="Wo_sb")

            def xtb_load(c0, c1):
                cs = slice(c0, c1)
                nc.sync.dma_start(
                    out=xTb_sb[:].rearrange("p (c t) -> p c t", c=8)[:, :, cs],
                    in_=xTb_t.rearrange("(c p) t -> p c t", p=P)[:, :, cs],
                )

            bias_tiles = {}

            def get_bias(b, sl, j4):
                """4-pair bias tile for pairs j4*4 .. j4*4+3 of slot (b, sl)."""
                key = (b, sl, j4)
                if key in bias_tiles:
                    return bias_tiles[key]
                ext = EXTA if sl == 0 else EXTB
                pair = j4 * 4 if sl == 0 else EXTA + j4 * 4
                npr = min(4, ext - j4 * 4)
                t = bpool.tile(
                    [P, 4 * HEADS * P], bf16, tag="bias", name=f"bias{b}{sl}{j4}"
                )
                nc.sync.dma_start(
                    out=t[:, 0 : npr * HEADS * P]
                    .rearrange("j (t hq) -> j t hq", t=npr),
                    in_=bias_t[b, pair : pair + npr]
                    .rearrange("t j h q -> j t (h q)"),
                )
                bias_tiles[key] = t
                return t

            # device drains in order: q-proj inputs first, then batch-0 x
            # pieces interleaved with the first bias tiles
            nc.sync.dma_start(
                out=xTo_sb[:].rearrange("p (c t) -> p c t", c=8),
                in_=xTo_t.rearrange("(c p) t -> p c t", p=P),
            )
            nc.sync.dma_start(
                out=Wq_sb[:].rearrange("p (c t) -> p c t", c=8),
                in_=Wq_t.rearrange("(c p) t -> p c t", p=P),
            )
            xtb_load(0, 512)  # kv window 0
            get_bias(0, 0, 0)
            xtb_load(512, 1024)  # kv window 1
            get_bias(0, 0, 1)
            get_bias(0, 1, 0)
            get_bias(0, 1, 1)
            xtb_load(1024, 2048)  # kv windows 2-3
            nc.sync.dma_start(
                out=Wo_sb[:].rearrange("p (c t) -> p c t", c=4),
                in_=Wo_t.rearrange("(c p) t -> p c t", p=P),
            )
            get_bias(0, 1, 2)
            xtb_load(2048, 2560)  # kv window 4
            get_bias(0, 1, 3)
            xtb_load(2560, 3072)  # kv window 5
            get_bias(1, 0, 0)
            xtb_load(3072, 3584)  # kv window 6
            xtb_load(3584, 4096)  # kv window 7

            # ---- k/v projection over ALL tokens, window by window ----
            # kvT_sb rows 0:64 = kT, rows 64:128 = vT (one copy per window)
            kvT_sb = cpool.tile([P, TOK_ALL], bf16, tag="kvT_sb")
            kT2 = kvT_sb[0:DH, :]
            vTs = kvT_sb[DH:P, :]
            V65 = cpool.tile([P, B * NT * (DH + 1)], bf16, tag="V65")
            nc.gpsimd.memset(V65[:, :], 1.0)

            def emit_kv(w):
                kvps = pspool.tile([P, 512], f32, tag="sT", name=f"kv{w}", bufs=st_bufs)
                for fc in range(8):
                    nc.tensor.matmul(
                        kvps[:, :],
                        Wkv_sb[:, fc * 2 * DH : (fc + 1) * 2 * DH],
                        xTb_sb[:, fc * TOK_ALL + w * 512 : fc * TOK_ALL + (w + 1) * 512],
                        start=(fc == 0),
                        stop=(fc == 7),
                    )
                nc.vector.tensor_copy(kvT_sb[:, w * 512 : (w + 1) * 512], kvps[:, :])

            def emit_tp(w):
                # transpose the window's 4 token-tiles into V65 (token-major)
                tp = pspool.tile([P, 4 * DH], bf16, tag="sT", name=f"tp{w}", bufs=st_bufs)
                for t4 in range(4):
                    t = w * 4 + t4
                    nc.tensor.matmul(
                        tp[:, t4 * DH : (t4 + 1) * DH],
                        vTs[:, t * P : (t + 1) * P],
                        ident_sb[DH:P, :],
                        is_transpose=True,
                        start=True,
                        stop=True,
                        skip_group_check=True,
                    )
                nc.vector.tensor_copy(
                    V65[:, w * 4 * (DH + 1) : (w * 4 + 4) * (DH + 1)]
                    .rearrange("p (t d) -> p t d", t=4)[:, :, 0:DH],
                    tp[:, :].rearrange("p (t d) -> p t d", t=4),
                )

            # ---- PE warmup: keep the clock ramped until Wq arrives ----
            warm = cpool.tile([P, 512], bf16, tag="warm")
            nc.gpsimd.memset(warm[:, :], 0.0)
            wps = pspool.tile([P, 512], f32, tag="pv", name="wps", bufs=pv_bufs)
            for i in range(30):
                nc.tensor.matmul(
                    wps[:, :], warm[:, 0:P], warm[:, :],
                    start=True, stop=True, skip_group_check=True,
                )

            # ---- q projection (own tokens), interleaved with kv w0-1 ----
            # qps tiles ride the pv ring (idle until the stream) so the sT
            # ring stays free for kv windows and stream scores
            qT_sb = cpool.tile([DH, HEADS * TOK_OWN], bf16, tag="qT_sb")

            def emit_q(hp):
                qps = pspool.tile([P, TOK_OWN], f32, tag="pv", name=f"qps{hp}", bufs=pv_bufs)
                for fc in range(8):
                    nc.tensor.matmul(
                        qps[:, :],
                        Wq_sb[:, fc * INNER + hp * P : fc * INNER + (hp + 1) * P],
                        xTo_sb[:, fc * TOK_OWN : (fc + 1) * TOK_OWN],
                        start=(fc == 0),
                        stop=(fc == 7),
                    )
                nc.vector.tensor_copy(
                    qT_sb[0:DH, (2 * hp) * TOK_OWN : (2 * hp + 1) * TOK_OWN],
                    qps[0:DH, :],
                )
                nc.vector.tensor_copy(
                    qT_sb[0:DH, (2 * hp + 1) * TOK_OWN : (2 * hp + 2) * TOK_OWN],
                    qps[DH:P, :],
                )

            emit_q(0)
            emit_q(1)
            emit_q(2)
            emit_q(3)
            emit_kv(0)
            emit_kv(1)
            emit_tp(0)
            emit_tp(1)
            qT3 = qT_sb[0:DH, :].rearrange("p (h t) -> p h t", h=HEADS)



            # ---- attention + output projection, software-pipelined ----
            slots = [(b, sl, ext) for b in range(B) for sl, ext in ((0, EXTA), (1, EXTB))]

            def make_tail(b, sl, pv):
                def tail():
                    # normalize: attnT = pv[0:64] * (1/l), l = pv[64]
                    # (reciprocal on DVE -- keeps the ACT table on Exp, no
                    #  LoadActFuncSet thrash blocking the stream's exps)
                    recip = cpool.tile([1, HEADS * P], f32, name=f"rc{b}{sl}", tag="recip", bufs=2)
                    nc.vector.reciprocal(recip[:, :], pv[DH : DH + 1, :])
                    bc_sb = ptpool.tile(
                        [DH, HEADS * P], f32, tag="pe", name=f"bc{b}{sl}"
                    )
                    nc.gpsimd.partition_broadcast(bc_sb[:, :], recip[:, :])
                    attnT = atpool.tile([P, HEADS * P], bf16, tag="at", name=f"at{b}{sl}")
                    nc.vector.tensor_mul(attnT[0:DH, :], pv[0:DH, :], bc_sb[:, :])
                    # shifted duplicate (rows 64:128 col g*128 hold head g+1)
                    # written directly from pv -- no dependent second copy
                    nc.vector.tensor_mul(
                        attnT[DH:P, 0 : 7 * P], pv[0:DH, P : HEADS * P],
                        bc_sb[:, P : HEADS * P],
                    )
                    # output projection for this slot's 128 tokens
                    # (one [P, 1024] PSUM tile: a single sT-ring slot, freed
                    #  by a single ob copy)
                    orow = (2 * b + sl) * P
                    ob_sb = obpool.tile([P, DIM], f32, tag="ob", name=f"ob{b}{sl}")
                    ops = pspool.tile(
                        [P, DIM], f32, tag="sT", name=f"op{b}{sl}", bufs=st_bufs
                    )
                    for half in range(2):
                        fs = slice(half * 512, (half + 1) * 512)
                        nc.tensor.matmul(
                            ops[:, fs], ones128[:, :], bo_sb[:, fs],
                            start=True, stop=False, skip_group_check=True,
                        )
                        for hp in range(4):
                            nc.tensor.matmul(
                                ops[:, fs],
                                attnT[:, 2 * hp * P : (2 * hp + 1) * P],
                                Wo_sb[:, hp * DIM + half * 512 : hp * DIM + (half + 1) * 512],
                                start=False,
                                stop=(hp == 3),
                                skip_group_check=True,
                            )
                        # copy+store per half so the out DMA overlaps the
                        # second half's matmuls (shortens the final tail)
                        nc.vector.tensor_copy(ob_sb[:, fs], ops[:, fs])
                        nc.scalar.dma_start(
                            out=out_t[orow : orow + P, fs], in_=ob_sb[:, fs]
                        )
                return tail

            for rep in range(reps):
                pending_tail = None
                # kv windows 2-7 are emitted between stream pairs, timed to
                # the xTb piece arrivals (PE has ~0.4us/pair of slack);
                # transposes follow 2 pairs later (ring slot frees quickly)
                splice_kv = {(0, 1): {3: 2, 6: 3, 10: 4, 13: 5}, (1, 0): {2: 6, 5: 7}}
                splice_tp = {(0, 1): {5: 2, 8: 3, 12: 4, 15: 5}, (1, 0): {4: 6, 7: 7}}
                for si, (b, sl, ext) in enumerate(slots):
                    qcol = (2 * b + sl) * P  # q columns in qT/attnT order
                    pv = pspool.tile(
                        [P, HEADS * P], f32, tag="pv", name=f"pv{b}{sl}",
                        bufs=pv_bufs,
                    )
                    bias_sb = None
                    pend_pv = []  # (jt, pt_sb) whose PV is not yet emitted

                    def emit_pv(jt, pt_sb):
                        g = (b * NT + jt) * (DH + 1)
                        for half in range(2):
                            nc.tensor.matmul(
                                pv[0 : DH + 1, half * 512 : (half + 1) * 512],
                                V65[:, g : g + DH + 1],
                                pt_sb[:, half * 512 : (half + 1) * 512],
                                start=(jt == 0),
                                stop=(jt == ext - 1),
                                skip_group_check=True,
                            )

                    for jt in range(ext):
                        pair = jt if sl == 0 else EXTA + jt
                        if jt % 4 == 0:
                            bias_sb = get_bias(b, sl, jt // 4)
                        bcol = (jt % 4) * HEADS * P
                        sT = pspool.tile(
                            [P, HEADS * P], f32, tag="sT", name=f"sT{b}{sl}{jt}",
                            bufs=st_bufs,
                        )
                        # scores: sT[:, half*512:+512] = kT.T @ qT (4 heads)
                        kcol = (b * NT + jt) * P
                        for half in range(2):
                            nc.tensor.matmul(
                                sT[:, half * 512 : (half + 1) * 512],
                                kT2[:, kcol : kcol + P],
                                qT3[:, 4 * half : 4 * half + 4, qcol : qcol + P],
                                start=True,
                                stop=True,
                                skip_group_check=True,
                            )
                        # e1 = exp(qk/8) straight from PSUM (ScalarE)
                        pe_sb = ptpool.tile(
                            [P, HEADS * P], bf16, tag="pe", name=f"pe{b}{sl}{jt}"
                        )
                        nc.scalar.activation(
                            pe_sb[:, :], sT[:, :], AF.Exp, scale=0.125
                        )
                        # P^T = e1 * exp(bias): pure-SBUF bf16 DVE multiply
                        # (4x SIMD); masked entries have exp(bias) == 0
                        pt_sb = ptpool.tile(
                            [P, HEADS * P], bf16, tag="pt", name=f"pt{b}{sl}{jt}",
                            bufs=5,
                        )
                        nc.vector.tensor_mul(
                            pt_sb[:, :], pe_sb[:, :],
                            bias_sb[:, bcol : bcol + HEADS * P],
                        )
                        # PV deferred 2 pairs: PE sees [scores,scores,PV,PV]
                        # bursts long enough to ramp to full clock
                        pend_pv.append((jt, pt_sb))
                        if jt % 2 == 1:
                            while len(pend_pv) > 2:
                                emit_pv(*pend_pv.pop(0))
                        if rep == 0:
                            w = splice_kv.get((b, sl), {}).get(jt)
                            if w is not None:
                                emit_kv(w)
                            w = splice_tp.get((b, sl), {}).get(jt)
                            if w is not None:
                                emit_tp(w)
                        if jt == tail_at and pending_tail is not None:
                            pending_tail()
                            pending_tail = None
                    while pend_pv:
                        emit_pv(*pend_pv.pop(0))
                    pending_tail = make_tail(b, sl, pv)
                pending_tail()

    nc.compile()
    return nc


def prep_inputs(x, rel_pos_bias, Wq, Wkv, Wo, bo):
    """Build the 8 per-core input maps (host-side sharding/marshalling)."""
    x = np.asarray(x, dtype=np.float32)
    rel_pos_bias = np.asarray(rel_pos_bias, dtype=np.float32)
    Wq = np.ascontiguousarray(np.asarray(Wq, dtype=np.float32))
    Wkv = np.ascontiguousarray(np.asarray(Wkv, dtype=np.float32))
    Wo = np.ascontiguousarray(np.asarray(Wo, dtype=np.float32))
    bo = np.asarray(bo, dtype=np.float32).reshape(1, DIM)
    ident = np.concatenate([np.eye(DH), np.eye(DH)], axis=0).astype(BF16)

    # full x, transposed, batch-major columns (col = b*2048 + n)
    xTb = np.ascontiguousarray(
        np.concatenate([x[b] for b in range(B)], axis=0).T
    ).astype(BF16)

    ji = np.arange(N)  # global key index
    in_maps = []
    for c in range(NCORES):
        tiles = _q_tiles(c)
        # own tokens, order [b0A, b0B, b1A, b1B]
        xs = [x[b, t * P : (t + 1) * P, :] for b in range(B) for t in tiles]
        xTo = np.ascontiguousarray(np.concatenate(xs, axis=0).T).astype(BF16)

        # expb = exp(bias), causal-masked entries exactly 0 (bias applied
        # multiplicatively after the on-device exp)
        biasT = np.zeros((B, NPAIR, P, HEADS, P), dtype=np.float32)
        for b in range(B):
            for sl, (t, ext) in enumerate(zip(tiles, (EXTA, EXTB))):
                qg = t * P + np.arange(P)  # global q index [128]
                ext_r = min(ext, t + 1)  # real causal extent in j-tiles
                nj = ext_r * P
                # [h, q, j] -> [jt, j, h, q]
                blk = rel_pos_bias[:, t * P : (t + 1) * P, :nj]
                blk = np.exp(blk.reshape(HEADS, P, ext_r, P)).transpose(2, 3, 0, 1)
                m = ji[:nj, None] > qg[None, :]  # [j, q] masked
                blk = np.where(
                    m.reshape(ext_r, P, 1, P).repeat(HEADS, axis=2),
                    0.0,
                    blk,
                )
                base = 0 if sl == 0 else EXTA
                biasT[b, base : base + ext_r] = blk
        in_maps.append(
            {
                "xTb": xTb,
                "xTo": xTo,
                "Wq": Wq.astype(BF16),
                "Wkv": Wkv.astype(BF16),
                "Wo": Wo.astype(BF16),
                "bo": bo,
                "ident": ident,
                "ones_r": np.ones((1, P), np.float32),
                "biasT": biasT.astype(BF16),
            }
        )
    return in_maps


def assemble(outs):
    """outs: list of 8 [512, 1024] arrays -> full [2, 2048, 1024]."""
    full = np.empty((B, N, DIM), dtype=np.float32)
    for c in range(NCORES):
        o = np.asarray(outs[c])
        for b in range(B):
            for sl, t in enumerate(_q_tiles(c)):
                full[b, t * P : (t + 1) * P, :] = o[(2 * b + sl) * P : (2 * b + sl + 1) * P]
    return full


def kernel(**inputs):
    from concourse.bass_utils import run_bass_kernel_spmd

    if "nc" not in _CACHE:
        _CACHE["nc"] = build_graph()
    nc = _CACHE["nc"]
    in_maps = prep_inputs(
        inputs["x"], inputs["rel_pos_bias"], inputs["Wq"], inputs["Wkv"],
        inputs["Wo"], inputs["bo"],
    )
    res = run_bass_kernel_spmd(
        nc, in_maps, core_ids=list(range(NCORES)),
        trace=bool(int(os.environ.get("KERNEL_TRACE", "0"))),
    )
    _CACHE["last_results"] = res
    return assemble([r["out"] for r in res.results])

